# revision 10
# baseline (speedup 1.0000x reference)
"""DFlashAttention kernel for Trainium2, tensor-parallel across 8 NeuronCores.

Sharding: Megatron-style head parallelism. Core c owns KV head c and Q heads
4c..4c+3 (matches repeat_interleave grouping), i.e. Wq rows [512c, 512c+512),
Wk/Wv rows [128c, 128c+128), Wo columns [512c, 512c+512). Each core computes a
partial output [QL, H]; the host sums the 8 partials (row-parallel Wo).

All layouts chosen so every matmul streams N>=256 columns (full-rate fp32r):
  - activations/weights fed feature-major (host pre-transposes)
  - Q/K kept d-major [HD, seq] for scores^T = K^T(dxk-tile) @ Q(dxq)
  - softmax over the partition axis: exp on ACT, k-tile reduction on DVE,
    cross-partition sum via ones-matmul, normalization via PE broadcast
  - V kept k-major [seq, HD] (PE-transposed after d-major projection), bf16,
    P^T bf16, so PV accumulates attn^T = V^T @ P^T in PSUM over 32 k-tiles
"""

import math
from contextlib import ExitStack

import numpy as np

import concourse.bass as bass
import concourse.bacc as bacc
import concourse.mybir as mybir
import concourse.tile as tile
from concourse.bass_utils import run_bass_kernel_spmd

F32 = mybir.dt.float32
F32R = mybir.dt.float32r
BF16 = mybir.dt.bfloat16
AF = mybir.ActivationFunctionType
ALU = mybir.AluOpType

# Full-problem dims (hardcoded per spec)
B, QL, CTX, H = 1, 2048, 2048, 4096
NH, NKV, HD = 32, 8, 128
NCORES = 8
HPC = NH // NKV  # 4 q-heads per core (one KV head per core)




def build_program(ql=QL, ctx_len=CTX, h=H, trace_sim=False):
    """Build the per-core Bass program (SPMD: same program, per-core shards)."""
    s = ql + ctx_len          # total kv length
    et = h // 128             # e-tiles (contraction tiles for projections)
    kt = s // 128             # k-tiles in attention
    QC = 512                  # phase A position-chunk
    nch = ql // QC            # chunks (ctx assumed == ql)
    assert ctx_len == ql, "phase A chunking assumes ctx_len == ql"
    QB = 512                  # phase B q-block
    nqb = ql // QB
    scale = 1.0 / math.sqrt(HD)
    DQ = HPC * HD             # 512: per-core q-head dim
    hot = h // 512            # output-column chunks in Wo stage

    nc = bacc.Bacc("TRN2", target_bir_lowering=False, debug=False)

    def din(name, shape, dt_=F32):
        return nc.dram_tensor(name, shape, dt_, kind="ExternalInput").ap()

    # matmul-feeding tensors are declared float32r (same bits as f32; the PE
    # rounds internally, and the BIR verifier wants the dtype consistent)
    hiddenT = din("hiddenT", [h, ql], F32R)      # hidden_states[0].T
    targetT = din("targetT", [h, ctx_len], F32R)  # target_hidden[0].T
    cosT = din("cosT", [HD, s])            # cos[0].T
    sinT = din("sinT", [HD, s])            # sign-folded sin[0].T
    wqT = din("wqT", [h, DQ], F32R)        # Wq[shard].T
    wkT = din("wkT", [h, HD], F32R)
    wvT = din("wvT", [h, HD], F32R)
    woT = din("woT", [DQ, h], F32R)        # Wo[:, shard].T
    ones_d = din("ones", [128, 128], F32R)
    ident_d = din("ident", [128, 128])
    out_d = nc.dram_tensor("out", [ql, h], F32, kind="ExternalOutput").ap()

    with tile.TileContext(nc, trace_sim=trace_sim) as tc, ExitStack() as ctx:
        persist = ctx.enter_context(tc.tile_pool(name="persist", bufs=1))
        ps = ctx.enter_context(
            tc.tile_pool(name="ps", bufs=8, space=bass.MemorySpace.PSUM)
        )

        qr_sb = persist.tile([128, HPC, ql], F32R, tag="qr")    # [d, h, q]
        kr_sb = persist.tile([128, s], F32R, tag="kr")          # [d, k]
        v_sb = persist.tile([128, kt, 128], BF16, tag="v")     # [k%128, ktile, d]
        ones_sb = persist.tile([128, 128], F32R, tag="ones")
        ident_sb = persist.tile([128, 128], F32, tag="ident")
        nc.sync.dma_start(ones_sb[:], ones_d[:])
        nc.sync.dma_start(ident_sb[:], ident_d[:])

        # ---------------- Phase A: projections + RoPE + V transpose ---------
        with (
            tc.tile_pool(name="wpool", bufs=1) as wpool,
            tc.tile_pool(name="apool", bufs=1) as apool,
        ):
            wq_sb = wpool.tile([128, et, DQ], F32R, tag="wq")   # [e%128, etile, d]
            wk_sb = wpool.tile([128, et, HD], F32R, tag="wk")
            wv_sb = wpool.tile([128, et, HD], F32R, tag="wv")
            nc.sync.dma_start(
                wq_sb[:], wqT.rearrange("(e p) d -> p e d", p=128)
            )
            nc.sync.dma_start(
                wk_sb[:], wkT.rearrange("(e p) d -> p e d", p=128)
            )
            nc.sync.dma_start(
                wv_sb[:], wvT.rearrange("(e p) d -> p e d", p=128)
            )

            def rope(ps_tile, cos_sl, sin_sl, dst):
                # dst = ps*cos + rot_half(ps)*sin  (sin sign pre-folded)
                raw = apool.tile([128, QC], F32, tag="rraw", bufs=2)
                nc.scalar.copy(raw[:], ps_tile[:])
                swp = apool.tile([128, QC], F32, tag="rswp", bufs=2)
                nc.sync.dma_start(swp[0:64, :], raw[64:128, :])
                nc.sync.dma_start(swp[64:128, :], raw[0:64, :])
                t1 = apool.tile([128, QC], F32, tag="rt1", bufs=2)
                nc.vector.tensor_tensor(t1[:], raw[:], cos_sl, ALU.mult)
                t2 = apool.tile([128, QC], F32, tag="rt2", bufs=2)
                nc.vector.tensor_tensor(t2[:], swp[:], sin_sl, ALU.mult)
                nc.vector.tensor_tensor(dst, t1[:], t2[:], ALU.add)

            for c in range(nch):
                q0 = c * QC
                cn = apool.tile([128, QC], F32, tag="cn", bufs=1)
                sn = apool.tile([128, QC], F32, tag="sn", bufs=1)
                cc = apool.tile([128, QC], F32, tag="cc", bufs=1)
                sc = apool.tile([128, QC], F32, tag="sc", bufs=1)
                nc.sync.dma_start(cn[:], cosT[:, ctx_len + q0:ctx_len + q0 + QC])
                nc.sync.dma_start(sn[:], sinT[:, ctx_len + q0:ctx_len + q0 + QC])
                nc.sync.dma_start(cc[:], cosT[:, q0:q0 + QC])
                nc.sync.dma_start(sc[:], sinT[:, q0:q0 + QC])

                psq = [
                    ps.tile([128, QC], F32, tag="ps", name=f"psq{i}")
                    for i in range(HPC)
                ]
                pskn = ps.tile([128, QC], F32, tag="ps")
                pskc = ps.tile([128, QC], F32, tag="ps")
                psvn = ps.tile([128, QC], F32, tag="ps")
                psvc = ps.tile([128, QC], F32, tag="ps")

                for e in range(et):
                    hs = apool.tile([128, QC], F32R, tag="hs", bufs=3)
                    nc.sync.dma_start(
                        hs[:], hiddenT[e * 128:e * 128 + 128, q0:q0 + QC]
                    )
                    ts_ = apool.tile([128, QC], F32R, tag="ts", bufs=3)
                    nc.sync.dma_start(
                        ts_[:], targetT[e * 128:e * 128 + 128, q0:q0 + QC]
                    )
                    st = dict(start=(e == 0), stop=(e == et - 1))
                    for hh in range(HPC):
                        nc.tensor.matmul(
                            psq[hh][:],
                            wq_sb[:, e, hh * 128:hh * 128 + 128],
                            hs[:],
                            **st,
                        )
                    nc.tensor.matmul(
                        pskn[:], wk_sb[:, e, :], hs[:], **st
                    )
                    nc.tensor.matmul(
                        psvn[:], wv_sb[:, e, :], hs[:], **st
                    )
                    nc.tensor.matmul(
                        pskc[:], wk_sb[:, e, :], ts_[:], **st
                    )
                    nc.tensor.matmul(
                        psvc[:], wv_sb[:, e, :], ts_[:], **st
                    )

                # RoPE: Q and K_noise at positions ctx+q0.., K_ctx at q0..
                for hh in range(HPC):
                    rope(psq[hh], cn[:], sn[:], qr_sb[:, hh, q0:q0 + QC])
                rope(pskn, cn[:], sn[:], kr_sb[:, ctx_len + q0:ctx_len + q0 + QC])
                rope(pskc, cc[:], sc[:], kr_sb[:, q0:q0 + QC])

                # V: d-major [d, k] chunks -> PE transpose -> k-major bf16
                for src, kbase in ((psvc, q0), (psvn, ctx_len + q0)):
                    vd = apool.tile([128, QC], F32, tag="vd", bufs=2)
                    nc.scalar.copy(vd[:], src[:])
                    for i in range(QC // 128):
                        pst = ps.tile([128, 128], F32, tag="ps")
                        nc.tensor.transpose(
                            pst[:], vd[:, i * 128:i * 128 + 128], ident_sb[:]
                        )
                        j = (kbase + i * 128) // 128
                        nc.scalar.copy(v_sb[:, j, :], pst[:])

        # ---------------- Phase B/C: attention + output projection ----------
        with tc.tile_pool(name="bpool", bufs=1) as bpool:
            wo_sb = bpool.tile([128, HPC, h], F32R, tag="wo")
            nc.sync.dma_start(
                wo_sb[:], woT.rearrange("(t p) o -> p t o", p=128)
            )

            for qb in range(nqb):
                qs0 = qb * QB
                ats = []
                for hh in range(HPC):
                    expst = bpool.tile([128, kt, QB], BF16, tag="expst", bufs=1)
                    for j in range(kt):
                        pss = ps.tile([128, QB], F32, tag="ps")
                        nc.tensor.matmul(
                            pss[:],
                            kr_sb[:, j * 128:j * 128 + 128],
                            qr_sb[:, hh, qs0:qs0 + QB],
                            start=True,
                            stop=True,
                        )
                        nc.scalar.activation(
                            expst[:, j, :], pss[:], AF.Exp, scale=scale
                        )
                    # attn^T accumulate over k-tiles
                    psat = ps.tile([128, QB], F32, tag="ps")
                    for j in range(kt):
                        nc.tensor.matmul(
                            psat[:],
                            v_sb[:, j, :],
                            expst[:, j, :],
                            start=(j == 0),
                            stop=(j == kt - 1),
                        )
                    # softmax denominator: sum over k = (sum over k-tiles) then
                    # cross-partition ones-matmul
                    part = bpool.tile([128, QB], F32R, tag="part", bufs=2)
                    with nc.allow_low_precision(
                        reason="f32r store of k-tile partial sums; PE rounds "
                        "to f32r at the ones-matmul anyway"
                    ):
                        nc.vector.tensor_reduce(
                            part[:],
                            expst.rearrange("p j q -> p q j"),
                            axis=mybir.AxisListType.X,
                            op=ALU.add,
                        )
                    psrs = ps.tile([1, QB], F32, tag="ps")
                    nc.tensor.matmul(
                        psrs[:], ones_sb[:, 0:1], part[:],
                        start=True, stop=True,
                    )
                    recip = bpool.tile([1, QB], F32R, tag="recip", bufs=2)
                    with nc.allow_low_precision(
                        reason="f32r reciprocal feeds the PE broadcast matmul"
                    ):
                        nc.vector.reciprocal(recip[:], psrs[:])
                    psb = ps.tile([128, QB], F32, tag="ps")
                    nc.tensor.matmul(
                        psb[:], ones_sb[0:1, :], recip[:],
                        start=True, stop=True,
                    )
                    at_raw = bpool.tile([128, QB], F32, tag="atraw", bufs=2)
                    nc.scalar.copy(at_raw[:], psat[:])
                    at_sb = bpool.tile([128, QB], F32R, tag="attnT", bufs=4)
                    nc.vector.tensor_tensor(at_sb[:], at_raw[:], psb[:], ALU.mult)
                    ats.append(at_sb)

                # Wo: out[q, ho] += attnT[t][:, q-tile].T @ woT[t][:, ho-chunk]
                for qs in range(QB // 128):
                    for oc in range(hot):
                        pso = ps.tile([128, 512], F32, tag="ps")
                        for t in range(HPC):
                            nc.tensor.matmul(
                                pso[:],
                                ats[t][:, qs * 128:qs * 128 + 128],
                                wo_sb[:, t, oc * 512:oc * 512 + 512],
                                start=(t == 0),
                                stop=(t == HPC - 1),
                            )
                        ob = bpool.tile([128, 512], F32, tag="ob", bufs=2)
                        if (qs + oc) % 2 == 0:
                            nc.scalar.copy(ob[:], pso[:])
                        else:
                            nc.vector.tensor_copy(ob[:], pso[:])
                        nc.sync.dma_start(
                            out_d[qs0 + qs * 128:qs0 + qs * 128 + 128,
                                  oc * 512:oc * 512 + 512],
                            ob[:],
                        )
    nc.compile()
    return nc


def make_in_maps(hidden_states, target_hidden, cos, sin, Wq, Wk, Wv, Wo):
    hidden_states = np.asarray(hidden_states, dtype=np.float32)
    target_hidden = np.asarray(target_hidden, dtype=np.float32)
    cos = np.asarray(cos, dtype=np.float32)
    sin = np.asarray(sin, dtype=np.float32)
    Wq = np.asarray(Wq, dtype=np.float32)
    Wk = np.asarray(Wk, dtype=np.float32)
    Wv = np.asarray(Wv, dtype=np.float32)
    Wo = np.asarray(Wo, dtype=np.float32)

    hT = np.ascontiguousarray(hidden_states[0].T)
    tT = np.ascontiguousarray(target_hidden[0].T)
    cT = np.ascontiguousarray(cos[0].T)
    sT = np.ascontiguousarray(sin[0].T).copy()
    sT[:64, :] *= -1.0  # fold rotate_half sign: rot(x)*sin == swap(x)*sT
    ident = np.eye(128, dtype=np.float32)
    ones = np.ones((128, 128), dtype=np.float32)

    in_maps = []
    for c in range(NCORES):
        in_maps.append({
            "hiddenT": hT,
            "targetT": tT,
            "cosT": cT,
            "sinT": sT,
            "wqT": np.ascontiguousarray(Wq[512 * c:512 * c + 512, :].T),
            "wkT": np.ascontiguousarray(Wk[128 * c:128 * c + 128, :].T),
            "wvT": np.ascontiguousarray(Wv[128 * c:128 * c + 128, :].T),
            "woT": np.ascontiguousarray(Wo[:, 512 * c:512 * c + 512].T),
            "ones": ones,
            "ident": ident,
        })
    return in_maps


_CACHE = {}
LAST_EXEC_NS = None
TRACE = False


def kernel(hidden_states, target_hidden, cos, sin, Wq, Wk, Wv, Wo):
    global LAST_EXEC_NS
    if "nc" not in _CACHE:
        _CACHE["nc"] = build_program()
    nc = _CACHE["nc"]
    in_maps = make_in_maps(
        hidden_states, target_hidden, cos, sin, Wq, Wk, Wv, Wo
    )
    res = run_bass_kernel_spmd(
        nc, in_maps, list(range(NCORES)), trace=TRACE
    )
    LAST_EXEC_NS = res.exec_time_ns
    out = np.zeros((QL, H), dtype=np.float32)
    for r in res.results:
        out += r["out"]
    return out.reshape(1, QL, H)



# revision 15
# speedup vs baseline: 1.0357x; 1.0357x over previous
"""DFlashAttention kernel for Trainium2, tensor-parallel across 8 NeuronCores.

Sharding: Megatron-style head parallelism. Core c owns KV head c and Q heads
4c..4c+3 (matches repeat_interleave grouping), i.e. Wq rows [512c, 512c+512),
Wk/Wv rows [128c, 128c+128), Wo columns [512c, 512c+512). Each core computes a
partial output [QL, H]; the host sums the 8 partials (row-parallel Wo).

All layouts chosen so every matmul streams N>=256 columns (full-rate fp32r):
  - activations/weights fed feature-major (host pre-transposes)
  - Q/K kept d-major [HD, seq] for scores^T = K^T(dxk-tile) @ Q(dxq)
  - softmax over the partition axis: exp on ACT, k-tile reduction on DVE,
    cross-partition sum via ones-matmul, normalization via PE broadcast
  - V kept k-major [seq, HD] (PE-transposed after d-major projection), bf16,
    P^T bf16, so PV accumulates attn^T = V^T @ P^T in PSUM over 32 k-tiles
"""

import math
from contextlib import ExitStack

import ml_dtypes
import numpy as np

import concourse.bass as bass
import concourse.bacc as bacc
import concourse.mybir as mybir
import concourse.tile as tile
from concourse.bass_utils import run_bass_kernel_spmd

F32 = mybir.dt.float32
F32R = mybir.dt.float32r
BF16 = mybir.dt.bfloat16
AF = mybir.ActivationFunctionType
ALU = mybir.AluOpType

# Full-problem dims (hardcoded per spec)
B, QL, CTX, H = 1, 2048, 2048, 4096
NH, NKV, HD = 32, 8, 128
NCORES = 8
HPC = NH // NKV  # 4 q-heads per core (one KV head per core)




def build_program(ql=QL, ctx_len=CTX, h=H, trace_sim=False, phases="ABC"):
    """Build the per-core Bass program (SPMD: same program, per-core shards)."""
    s = ql + ctx_len          # total kv length
    et = h // 128             # e-tiles (contraction tiles for projections)
    kt = s // 128             # k-tiles in attention
    QC = 512                  # phase A position-chunk
    nch = ql // QC            # chunks (ctx assumed == ql)
    assert ctx_len == ql, "phase A chunking assumes ctx_len == ql"
    QB = 512                  # phase B q-block
    nqb = ql // QB
    scale = 1.0 / math.sqrt(HD)
    DQ = HPC * HD             # 512: per-core q-head dim
    hot = h // 512            # output-column chunks in Wo stage

    nc = bacc.Bacc("TRN2", target_bir_lowering=False, debug=False)

    def din(name, shape, dt_=F32):
        return nc.dram_tensor(name, shape, dt_, kind="ExternalInput").ap()

    # matmul-feeding tensors are declared float32r (same bits as f32; the PE
    # rounds internally, and the BIR verifier wants the dtype consistent)
    hiddenT = din("hiddenT", [h, ql], BF16)      # hidden_states[0].T
    targetT = din("targetT", [h, ctx_len], BF16)  # target_hidden[0].T
    cosT = din("cosT", [HD, s])            # cos[0].T
    sinT = din("sinT", [HD, s])            # sign-folded sin[0].T
    wqT = din("wqT", [h, DQ], BF16)        # Wq[shard].T
    wkT = din("wkT", [h, HD], BF16)
    wvT = din("wvT", [h, HD], BF16)
    woT = din("woT", [DQ, h], F32R)        # Wo[:, shard].T
    ones_d = din("ones", [128, 128], F32R)
    onesb_d = din("ones_bf", [128, 1], BF16)
    ident_d = din("ident", [128, 128])
    out_d = nc.dram_tensor("out", [ql, h], F32, kind="ExternalOutput").ap()

    with tile.TileContext(nc, trace_sim=trace_sim) as tc, ExitStack() as ctx:
        persist = ctx.enter_context(tc.tile_pool(name="persist", bufs=1))
        ps = ctx.enter_context(
            tc.tile_pool(name="ps", bufs=8, space=bass.MemorySpace.PSUM)
        )

        qr_sb = persist.tile([128, HPC, ql], F32R, tag="qr")    # [d, h, q]
        kr_sb = persist.tile([128, s], F32R, tag="kr")          # [d, k]
        v_sb = persist.tile([128, kt, 128], BF16, tag="v")     # [k%128, ktile, d]
        ones_sb = persist.tile([128, 128], F32R, tag="ones")
        onesb_sb = persist.tile([128, 1], BF16, tag="onesb")
        ident_sb = persist.tile([128, 128], F32, tag="ident")
        nc.sync.dma_start(ones_sb[:], ones_d[:])
        nc.sync.dma_start(onesb_sb[:], onesb_d[:])
        nc.sync.dma_start(ident_sb[:], ident_d[:])

        # ---------------- Phase A: projections + RoPE + V transpose ---------
        with (
            tc.tile_pool(name="wpool", bufs=1) as wpool,
            tc.tile_pool(name="apool", bufs=1) as apool,
        ):
            wq_sb = wpool.tile([128, et, DQ], BF16, tag="wq")   # [e%128, etile, d]
            wk_sb = wpool.tile([128, et, HD], BF16, tag="wk")
            wv_sb = wpool.tile([128, et, HD], BF16, tag="wv")
            nc.sync.dma_start(
                wq_sb[:], wqT.rearrange("(e p) d -> p e d", p=128)
            )
            nc.sync.dma_start(
                wk_sb[:], wkT.rearrange("(e p) d -> p e d", p=128)
            )
            nc.sync.dma_start(
                wv_sb[:], wvT.rearrange("(e p) d -> p e d", p=128)
            )

            def rope(ps_tile, cos_sl, sin_sl, dst):
                # dst = ps*cos + rot_half(ps)*sin  (sin sign pre-folded)
                raw = apool.tile([128, QC], F32, tag="rraw", bufs=3)
                nc.scalar.copy(raw[:], ps_tile[:])
                swp = apool.tile([128, QC], F32, tag="rswp", bufs=3)
                nc.sync.dma_start(swp[0:64, :], raw[64:128, :])
                nc.sync.dma_start(swp[64:128, :], raw[0:64, :])
                t1 = apool.tile([128, QC], F32, tag="rt1", bufs=2)
                nc.vector.tensor_tensor(t1[:], raw[:], cos_sl, ALU.mult)
                t2 = apool.tile([128, QC], F32, tag="rt2", bufs=2)
                nc.vector.tensor_tensor(t2[:], swp[:], sin_sl, ALU.mult)
                nc.vector.tensor_tensor(dst, t1[:], t2[:], ALU.add)

            for c in range(nch):
                q0 = c * QC
                cn = apool.tile([128, QC], F32, tag="cn", bufs=1)
                sn = apool.tile([128, QC], F32, tag="sn", bufs=1)
                cc = apool.tile([128, QC], F32, tag="cc", bufs=1)
                sc = apool.tile([128, QC], F32, tag="sc", bufs=1)
                nc.sync.dma_start(cn[:], cosT[:, ctx_len + q0:ctx_len + q0 + QC])
                nc.sync.dma_start(sn[:], sinT[:, ctx_len + q0:ctx_len + q0 + QC])
                nc.sync.dma_start(cc[:], cosT[:, q0:q0 + QC])
                nc.sync.dma_start(sc[:], sinT[:, q0:q0 + QC])

                psq = [
                    ps.tile([128, QC], F32, tag="ps", name=f"psq{i}")
                    for i in range(HPC)
                ]
                pskn = ps.tile([128, QC], F32, tag="ps")
                pskc = ps.tile([128, QC], F32, tag="ps")
                psvn = ps.tile([128, QC], F32, tag="ps")
                psvc = ps.tile([128, QC], F32, tag="ps")

                for e in range(et):
                    hs = apool.tile([128, QC], BF16, tag="hs", bufs=6)
                    nc.sync.dma_start(
                        hs[:], hiddenT[e * 128:e * 128 + 128, q0:q0 + QC]
                    )
                    ts_ = apool.tile([128, QC], BF16, tag="ts", bufs=6)
                    nc.sync.dma_start(
                        ts_[:], targetT[e * 128:e * 128 + 128, q0:q0 + QC]
                    )
                    st = dict(start=(e == 0), stop=(e == et - 1))
                    for hh in range(HPC):
                        nc.tensor.matmul(
                            psq[hh][:],
                            wq_sb[:, e, hh * 128:hh * 128 + 128],
                            hs[:],
                            **st,
                        )
                    nc.tensor.matmul(
                        pskn[:], wk_sb[:, e, :], hs[:], **st
                    )
                    nc.tensor.matmul(
                        psvn[:], wv_sb[:, e, :], hs[:], **st
                    )
                    nc.tensor.matmul(
                        pskc[:], wk_sb[:, e, :], ts_[:], **st
                    )
                    nc.tensor.matmul(
                        psvc[:], wv_sb[:, e, :], ts_[:], **st
                    )

                # RoPE: Q and K_noise at positions ctx+q0.., K_ctx at q0..
                for hh in range(HPC):
                    rope(psq[hh], cn[:], sn[:], qr_sb[:, hh, q0:q0 + QC])
                rope(pskn, cn[:], sn[:], kr_sb[:, ctx_len + q0:ctx_len + q0 + QC])
                rope(pskc, cc[:], sc[:], kr_sb[:, q0:q0 + QC])

                # V: d-major [d, k] chunks -> PE transpose -> k-major bf16
                for src, kbase in ((psvc, q0), (psvn, ctx_len + q0)):
                    vd = apool.tile([128, QC], F32, tag="vd", bufs=2)
                    nc.scalar.copy(vd[:], src[:])
                    for i in range(QC // 128):
                        pst = ps.tile([128, 128], F32, tag="ps")
                        nc.tensor.transpose(
                            pst[:], vd[:, i * 128:i * 128 + 128], ident_sb[:]
                        )
                        j = (kbase + i * 128) // 128
                        nc.scalar.copy(v_sb[:, j, :], pst[:])

        # ---------------- Phase B/C: attention + output projection ----------
        with tc.tile_pool(name="bpool", bufs=1) as bpool:
            wo_sb = bpool.tile([128, HPC, h], F32R, tag="wo")
            nc.sync.dma_start(
                wo_sb[:], woT.rearrange("(t p) o -> p t o", p=128)
            )

            for qb in range(nqb if "B" in phases else 0):
                qs0 = qb * QB
                ats = []
                for hh in range(HPC):
                    expst = bpool.tile([128, kt, QB], BF16, tag="expst", bufs=1)
                    for j in range(kt):
                        pss = ps.tile([128, QB], F32, tag="ps")
                        nc.tensor.matmul(
                            pss[:],
                            kr_sb[:, j * 128:j * 128 + 128],
                            qr_sb[:, hh, qs0:qs0 + QB],
                            start=True,
                            stop=True,
                        )
                        nc.scalar.activation(
                            expst[:, j, :], pss[:], AF.Exp, scale=scale
                        )
                    # attn^T accumulate over k-tiles; rowsum accumulated on
                    # PE in parallel (stationary ones-column, one bank)
                    psat = ps.tile([128, QB], F32, tag="ps")
                    psrs = ps.tile([1, QB], F32, tag="ps")
                    for j in range(kt):
                        nc.tensor.matmul(
                            psat[:],
                            v_sb[:, j, :],
                            expst[:, j, :],
                            start=(j == 0),
                            stop=(j == kt - 1),
                        )
                        nc.tensor.matmul(
                            psrs[:], onesb_sb[:], expst[:, j, :],
                            start=(j == 0), stop=(j == kt - 1),
                        )
                    recip = bpool.tile([1, QB], F32R, tag="recip", bufs=2)
                    with nc.allow_low_precision(
                        reason="f32r reciprocal feeds the PE broadcast matmul"
                    ):
                        nc.vector.reciprocal(recip[:], psrs[:])
                    psb = ps.tile([128, QB], F32, tag="ps")
                    nc.tensor.matmul(
                        psb[:], ones_sb[0:1, :], recip[:],
                        start=True, stop=True,
                    )
                    at_raw = bpool.tile([128, QB], F32, tag="atraw", bufs=2)
                    nc.scalar.copy(at_raw[:], psat[:])
                    at_sb = bpool.tile([128, QB], F32R, tag="attnT", bufs=4)
                    nc.vector.tensor_tensor(at_sb[:], at_raw[:], psb[:], ALU.mult)
                    ats.append(at_sb)

                # Wo: out[q, ho] += attnT[t][:, q-tile].T @ woT[t][:, ho-chunk]
                for qs in range(QB // 128 if "C" in phases else 0):
                    for oc in range(hot):
                        pso = ps.tile([128, 512], F32, tag="ps")
                        for t in range(HPC):
                            nc.tensor.matmul(
                                pso[:],
                                ats[t][:, qs * 128:qs * 128 + 128],
                                wo_sb[:, t, oc * 512:oc * 512 + 512],
                                start=(t == 0),
                                stop=(t == HPC - 1),
                            )
                        ob = bpool.tile([128, 512], F32, tag="ob", bufs=2)
                        if (qs + oc) % 2 == 0:
                            nc.scalar.copy(ob[:], pso[:])
                        else:
                            nc.vector.tensor_copy(ob[:], pso[:])
                        nc.sync.dma_start(
                            out_d[qs0 + qs * 128:qs0 + qs * 128 + 128,
                                  oc * 512:oc * 512 + 512],
                            ob[:],
                        )
    return _finish(nc)


def _finish(nc):
    nc.compile()
    return nc


def make_in_maps(hidden_states, target_hidden, cos, sin, Wq, Wk, Wv, Wo):
    hidden_states = np.asarray(hidden_states, dtype=np.float32)
    target_hidden = np.asarray(target_hidden, dtype=np.float32)
    cos = np.asarray(cos, dtype=np.float32)
    sin = np.asarray(sin, dtype=np.float32)
    Wq = np.asarray(Wq, dtype=np.float32)
    Wk = np.asarray(Wk, dtype=np.float32)
    Wv = np.asarray(Wv, dtype=np.float32)
    Wo = np.asarray(Wo, dtype=np.float32)

    bf16 = ml_dtypes.bfloat16
    hT = np.ascontiguousarray(hidden_states[0].T).astype(bf16)
    tT = np.ascontiguousarray(target_hidden[0].T).astype(bf16)
    cT = np.ascontiguousarray(cos[0].T)
    sT = np.ascontiguousarray(sin[0].T).copy()
    sT[:64, :] *= -1.0  # fold rotate_half sign: rot(x)*sin == swap(x)*sT
    ident = np.eye(128, dtype=np.float32)
    ones = np.ones((128, 128), dtype=np.float32)

    in_maps = []
    for c in range(NCORES):
        in_maps.append({
            "hiddenT": hT,
            "targetT": tT,
            "cosT": cT,
            "sinT": sT,
            "wqT": np.ascontiguousarray(Wq[512 * c:512 * c + 512, :].T).astype(bf16),
            "wkT": np.ascontiguousarray(Wk[128 * c:128 * c + 128, :].T).astype(bf16),
            "wvT": np.ascontiguousarray(Wv[128 * c:128 * c + 128, :].T).astype(bf16),
            "woT": np.ascontiguousarray(Wo[:, 512 * c:512 * c + 512].T),
            "ones": ones,
            "ones_bf": np.ones((128, 1), dtype=bf16),
            "ident": ident,
        })
    return in_maps


_CACHE = {}
LAST_EXEC_NS = None
TRACE = False


def kernel(hidden_states, target_hidden, cos, sin, Wq, Wk, Wv, Wo):
    global LAST_EXEC_NS
    if "nc" not in _CACHE:
        _CACHE["nc"] = build_program()
    nc = _CACHE["nc"]
    in_maps = make_in_maps(
        hidden_states, target_hidden, cos, sin, Wq, Wk, Wv, Wo
    )
    res = run_bass_kernel_spmd(
        nc, in_maps, list(range(NCORES)), trace=TRACE
    )
    LAST_EXEC_NS = res.exec_time_ns
    out = np.zeros((QL, H), dtype=np.float32)
    for r in res.results:
        out += r["out"]
    return out.reshape(1, QL, H)

